# revision 15
# baseline (speedup 1.0000x reference)
"""Trainium2 Bass kernel for nn_Always (sliding-window smoothed-min).

The reference "scan" is a sliding-window reduction:
    out[b, t, d] = -(1/5) * log( sum_{k=0..15} exp(-5 * x[b, t-k, d]) )
with x[b, j, d] := x[b, 0, d] for j < 0 (the h0 padding).

Strategy (pure data parallel over 8 cores; 2 batches x 2 tensors per core):
  - Device input is bf16 and HOST-PERMUTED into the compute layout, so
    every DMA descriptor is a 1-4 KB contiguous run (vs 256 B in the
    naive [t, d] layout). The host does x[b].reshape(64, 128, 64)
    .transpose(1, 0, 2): partition p holds timesteps t = 128*J + p,
    free axis is (J, d). The inverse permute runs on the host after.
  - VectorE (DVE): E = exp(-5x) via a Schraudolph bit-trick entirely in
    16-bit: i16 = round(A*x + B) reinterpreted as bf16 gives 2^(A'x+B')
    with ~+-9% worst-case rel err (incl. bf16 input rounding), which the
    smoothed-min output absorbs to ~3e-3 l2 rel err (tolerance is 2e-2).
    This moves exp off the Scalar engine, whose ln throughput (together
    with TensorE) paces the pipeline.
  - TensorE: banded matmuls (bf16) compute the 16-wide window sum S.
    R=1 layout means ONE in-band matrix W_in (po-pi in [0,15]) and one
    halo matrix W_halo (reads the previous 128-step tile via a shifted
    view of the same buffer -- no copies), W_first handles t<16 padding.
  - ScalarE: only ln(S) from PSUM -> bf16 (one ACT table set, no swaps).
  - Output ships as int8: q = round(4 * ln S) (|ln S| < 31.75 here), the
    host computes q * (1/4) * (-1/5) in f32. Quantization adds ~0.8e-2
    l2 rel err, within the 2e-2 budget, and halves output HBM traffic.
  - Output DMAs are gated behind the 6th ln: the SDMA engines round-robin
    rings at packet granularity, so an early output DMA would halve the
    input stream bandwidth and starve the pipeline of late sequences.
"""

import numpy as np

B, T, D = 16, 8192, 64
N_CORES = 8
B_PER_CORE = B // N_CORES  # 2
SCALE = 5.0
WIN = 16
P = 128                    # SBUF partitions; tile = 128 timesteps (R=1)
SEQ_TILES = T // P         # 64 tiles per sequence
SEQ_COLS = SEQ_TILES * D   # 4096 free columns per sequence
N_SEQS = 2 * B_PER_CORE    # 4 sequences per core (2 tensors x 2 batches)
QT = 8                     # tiles per PSUM bank (matmul granularity)
QCOLS = QT * D             # 512 cols = 2 KB f32 = 1 bank

# Schraudolph exp constants: i16 = A*x + B, bits(i16) read as bf16
# approximates 2^(-5*log2(e)*x). c=0.0579 centers the linear-mantissa err.
EXP_A = float(-5.0 * np.log2(np.e) * 128.0)
EXP_B = float(128.0 * (127.0 - 0.0579))
OUT_SC = 4.0               # int8 output quantization scale


def _weight_mats():
    """[128, 384] bf16: W_in | W_halo | W_first.
    Layout convention: lhsT[p_in, p_out]; matmul computes lhsT.T @ rhs."""
    import ml_dtypes

    p = np.arange(P)
    dd = p[None, :] - p[:, None]  # p_out - p_in
    w_in = ((dd >= 0) & (dd <= WIN - 1)).astype(np.float32)
    # halo: input from previous tile, dd_eff = dd + 128 in [1, 15]
    w_halo = ((dd + P >= 1) & (dd + P <= WIN - 1)).astype(np.float32)
    # first tile of a sequence: taps at t<0 all read x[0] (partition 0)
    w_first = np.zeros((P, P), np.float32)
    w_first[0, :] = np.maximum(WIN - 1 - p, 0)
    return np.concatenate([w_in, w_halo, w_first], axis=1).astype(
        ml_dtypes.bfloat16
    )


# per-seq plans: input/exp part sizes (cols), chunk list (tile0, ntiles),
# and out-DMA grouping (chunks per DMA). Sequence 0 is finer so the first
# ln fires as early as possible; later sequences use full-size granules.
def _seq_plan(s):
    if s == 0:
        return (
            [512, 512, 1024, 1024, 1024],
            [(0, 8), (8, 8), (16, 16), (32, 16), (48, 16)],
            [2, 2, 1],
        )
    return (
        [2048, 2048],
        [(0, 16), (16, 16), (32, 16), (48, 16)],
        [2, 2],
    )


def _build_bass(mode="grouped"):
    from contextlib import ExitStack

    import concourse.bacc as bacc
    import concourse.tile as tile
    from concourse import mybir
    from concourse.tile import add_dep_helper

    f32 = mybir.dt.float32
    bf16 = mybir.dt.bfloat16
    i16 = mybir.dt.int16
    i8 = mybir.dt.int8
    AF = mybir.ActivationFunctionType
    ALU = mybir.AluOpType

    nc = bacc.Bacc(trn_type="TRN2")
    xin = nc.dram_tensor("xin", [N_SEQS, P, SEQ_COLS], bf16, kind="ExternalInput")
    yout = nc.dram_tensor("yout", [N_SEQS, P, SEQ_COLS], i8, kind="ExternalOutput")
    w_all_d = nc.inline_tensor(_weight_mats(), name="w_all_c")

    with tile.TileContext(nc) as tc, ExitStack() as ctx:
        consts = ctx.enter_context(tc.tile_pool(name="consts", bufs=1))
        x_pool = ctx.enter_context(tc.tile_pool(name="x", bufs=N_SEQS))
        e_pool = ctx.enter_context(tc.tile_pool(name="e", bufs=N_SEQS))
        o_pool = ctx.enter_context(tc.tile_pool(name="o", bufs=8))
        o8_pool = ctx.enter_context(tc.tile_pool(name="o8", bufs=8))
        ps_pool = ctx.enter_context(tc.tile_pool(name="ps", bufs=4, space="PSUM"))

        w_all = consts.tile([P, 3 * P], bf16)
        W_IN = w_all[:, 0:P]
        W_HALO = w_all[:, P : 2 * P]
        W_FIRST = w_all[:, 2 * P : 3 * P]

        # scalar ring: its descriptor-gen overlaps the first input DMA's
        nc.scalar.dma_start(w_all[:], w_all_d[:])

        # ---- input DMAs, all emitted first on the SP sequencer
        xts = []
        for s in range(N_SEQS):
            parts, _, _ = _seq_plan(s)
            xt = x_pool.tile([P, SEQ_COLS], bf16)
            c0 = 0
            for w in parts:
                nc.sync.dma_start(xt[:, c0 : c0 + w], xin[s][:, c0 : c0 + w])
                c0 += w
            xts.append(xt)

        # ---- DVE exp (all emitted before the int8 converts in DVE order)
        ets = []
        for s in range(N_SEQS):
            parts, _, _ = _seq_plan(s)
            et = e_pool.tile([P, SEQ_COLS], bf16)
            c0 = 0
            for w in parts:
                nc.vector.tensor_scalar(
                    et[:, c0 : c0 + w].bitcast(i16),
                    xts[s][:, c0 : c0 + w],
                    EXP_A,
                    EXP_B,
                    op0=ALU.mult,
                    op1=ALU.add,
                )
                c0 += w
            ets.append(et)

        # ---- matmul window-sums + ln + int8 quant + output DMA
        n_ln = 0
        ln_gate = None
        out_insts = []
        for s in range(N_SEQS):
            _, chunks, groups = _seq_plan(s)
            et3 = ets[s][:].rearrange("p (J d) -> p J d", d=D)
            gi, gleft, o8, gfill, g0col = 0, groups[0], None, 0, 0
            for ci, (t0, nt) in enumerate(chunks):
                ps = ps_pool.tile([P, nt * D], f32)
                for m in range(nt // QT):
                    J0 = t0 + m * QT
                    outp = ps[:, m * QCOLS : (m + 1) * QCOLS]
                    nc.tensor.matmul(
                        outp, W_IN, et3[:, J0 : J0 + QT, :], start=True, stop=False
                    )
                    if J0 == 0:
                        # no previous tile: tiles 0..6 feed out-tiles 1..7;
                        # the t<16 padding taps come from W_first
                        nc.tensor.matmul(
                            ps[:, D:QCOLS], W_HALO, et3[:, 0 : QT - 1, :],
                            start=False, stop=False,
                        )
                        nc.tensor.matmul(
                            ps[:, 0:D], W_FIRST, et3[:, 0:1, :],
                            start=False, stop=True,
                        )
                    else:
                        nc.tensor.matmul(
                            outp, W_HALO, et3[:, J0 - 1 : J0 + QT - 1, :],
                            start=False, stop=True,
                        )
                ot = o_pool.tile([P, nt * D], bf16)
                ln_i = nc.scalar.activation(ot[:], ps[:], AF.Ln).ins
                n_ln += 1
                if n_ln == 6:
                    ln_gate = ln_i
                if o8 is None:
                    gcols = sum(chunks[ci + k][1] for k in range(gleft)) * D
                    o8 = o8_pool.tile([P, gcols], i8)
                    gfill, g0col = 0, t0 * D
                nc.vector.tensor_scalar_mul(
                    o8[:, gfill : gfill + nt * D], ot[:], OUT_SC
                )
                gfill += nt * D
                gleft -= 1
                if gleft == 0:
                    out_insts.append(
                        nc.sync.dma_start(
                            yout[s][:, g0col : g0col + gfill], o8[:]
                        ).ins
                    )
                    gi += 1
                    gleft = groups[gi] if gi < len(groups) else 0
                    o8 = None

        # outputs yield the SDMA engines to the input stream: engines
        # round-robin rings at packet granularity, so an early output DMA
        # would halve the input bandwidth and starve the exp->matmul->ln
        # pipeline of late sequences. Gate the early out groups on the 6th
        # ln, which fires roughly as the input stream drains. (A direct
        # DMA-on-DMA semaphore dep hangs the HW.)
        for out_i in out_insts[:4]:
            add_dep_helper(out_i, ln_gate, sync=True, reason="ins first")
    nc.compile()
    return nc


def _permute_in(x):
    """[B, T, D] f32 -> [B, P, SEQ_COLS] bf16 with t = 128*J + p."""
    import ml_dtypes

    return np.ascontiguousarray(
        np.asarray(x, dtype=np.float32)
        .reshape(B, SEQ_TILES, P, D)
        .transpose(0, 2, 1, 3)
        .reshape(B, P, SEQ_COLS)
    ).astype(ml_dtypes.bfloat16)


def _permute_out(y):
    """[P, SEQ_COLS] int8 -> [T, D] f32 (dequant + inverse permute)."""
    return (
        np.asarray(y).astype(np.float32) * (-1.0 / (SCALE * OUT_SC))
    ).reshape(P, SEQ_TILES, D).transpose(1, 0, 2).reshape(T, D)


def _run(lower_trace, upper_trace, trace=False, mode="grouped", **spmd_kwargs):
    from concourse.bass_utils import run_bass_kernel_spmd

    lp = _permute_in(lower_trace)
    up = _permute_in(upper_trace)

    nc = _build_bass(mode=mode)
    in_maps = []
    for i in range(N_CORES):
        b0, b1 = 2 * i, 2 * i + 1
        in_maps.append(
            {"xin": np.ascontiguousarray(np.stack([lp[b0], lp[b1], up[b0], up[b1]]))}
        )
    res = run_bass_kernel_spmd(
        nc, in_maps, core_ids=list(range(N_CORES)), trace=trace, **spmd_kwargs
    )
    out_lower = np.empty((B, T, D), np.float32)
    out_upper = np.empty((B, T, D), np.float32)
    for i in range(N_CORES):
        y = res.results[i]["yout"]
        out_lower[2 * i] = _permute_out(y[0])
        out_lower[2 * i + 1] = _permute_out(y[1])
        out_upper[2 * i] = _permute_out(y[2])
        out_upper[2 * i + 1] = _permute_out(y[3])
    return (out_lower, out_upper), res


def kernel(lower_trace, upper_trace):
    (out_lower, out_upper), _ = _run(lower_trace, upper_trace, trace=False)
    return out_lower, out_upper
